# revision 46
# baseline (speedup 1.0000x reference)
"""Trainium2 Bass kernel for BSplineBasis (degree-3, 64 uniform-ish knots).

Math: the reference evaluates, for each normalized point xn and each of 60
basis elements i, a piecewise cubic (de Boor with clamped interval index).
With simple inner knots this is exactly representable in truncated-power form:

    out[n, i] = sum_q A[q,i] * y^q  +  sum_m J[m,i] * relu(y - kap'_m)^3

with y = xn - 0.5 (centered to tame monomial cancellation at tf32 precision),
56 inner-knot features kap' and a banded (3 taps/column) jump table J.

Device pipeline per core (62,720-point shard = 128 tiles of 490 points,
pair u = tiles (u, 64+u) packed block-diagonally so matmuls use 120 output
partitions):
  phase A: global min/max of the full x (redundant per core -- the model's
      AllReduce costs 28us, far more than the 5.7us redundant read); all
      input DMAs share the sync queue so the 2MB min/max stream is not
      delayed by the table loads behind it; chunked DMA overlapped with DVE
      min / Pool XYZWC-max reduces, small final chunk shortens the tail;
      a partition_all_reduce finalize leaves (-min, max) on every partition
      so the y pass reads the derived (inv, q) scalars straight from SBUF,
      no broadcast matmul
  xp (DVE): fp16 power table [y | y^2 | y^3] [128, 3*490]; tiny PE warmup
      matmuls throughout phase A keep the tensor engine's p-state clock
      ramped so the main loop starts at the full 2.4 GHz
  xr6: three strided DMAs, each issued as soon as its power row is ready
      (y on the Pool SWDGE queue, y^2 on SP, y^3 on the ACT queue),
      redistribute xp into matmul-K layout [6, 64*490] fp16,
      row 2q+h = power q+1 of half h
  iterations 0-1 bridge the ~2.4us xr6 DMA latency: pairs 0, 32, 1, 33 run
      MM1 straight off xp with three accumulating K=66/K=98 matmuls whose
      stationary weights are one-hot row-selectors (the PE requires
      contiguous partitions from base 0, so the selector picks the pair's
      rows a and 64+a out of a base-0 window just wide enough to reach
      them; matmul cost only depends on N, so wide K costs the same as K=6)
  loop over 32 iters (2 pairs each), software-pipelined: PE runs MM1 two
      iterations ahead, MM2 reuses MM1's PSUM banks (the psum pool then
      holds 4 iterations in flight on the 8 banks):
    MM1 (K=6, fp16): cube polynomials + y-power pass-throughs -> PSUM
    clamp (ACT): stack = Relu(G + bias) -> fp16, per-partition bias AP;
      power rows are nonneg-shifted so plain Relu is exact (shift
      corrections folded into the MM2 weights); values carry a x16 scale
      (W2 /16) so everything sits comfortably in fp16 range
    MM2 (K=120, fp16): blockdiag(W2, W2).T @ stack -> PSUM
    evict (DVE): PSUM -> bf16 stage; the iteration-30 evict runs on ACT
      after its clamps finish (mid-stream ACT evicts stall the DVE stream;
      a single tail evict on ACT lets both engines finish together)
  out DMA per pair-couple: stage [120, 1960] bf16 -> out_t (contiguous);
  the last four iterations stage per-iter so the tail DMAs are half-sized
  and spread over the bus
Host: out_t [120, 31360] -> [62720, 60] by reshape/transpose (pair order
below); the (rare) xn == 1.0 rows are patched to the reference's degenerate
right-end pieces. Output travels as bf16 (l2-rel ~2e-3, tolerance 2e-2).

Tables are derived from the knots at runtime on the host in float64, by
fitting the reference's own de Boor piece recursion (exact for cubics).
"""
import os
import sys

import numpy as np

if "/opt/trn_rl_repo" not in sys.path:
    sys.path.insert(0, "/opt/trn_rl_repo")

DEGREE = 3
NUM_KNOTS = 64
NB = NUM_KNOTS - DEGREE - 1          # 60 basis elements
N_POINTS = 500_000
N_CORES = 8
SHARD = N_POINTS // N_CORES          # 62500
TILE_W = 490                          # points per matmul tile
N_TILES = 128                         # 128 * 490 = 62720 >= SHARD
SHARD_PAD = N_TILES * TILE_W          # 62720
N_PAIRS = 64                          # pair u = tiles (u, 64+u)
HALF = SHARD_PAD // 2                 # 31360
NF = 56                               # truncated-power features
FULL_COLS = 3907                      # 128 * 3907 = 500096 >= N_POINTS
FULL_PAD = 128 * FULL_COLS
CENTER = 0.5
SHIFT1 = 0.5                          # nonneg shift for the y row (-> xn)
SHIFT3 = 0.125                        # nonneg shift for the y^3 row

# processing order: iteration i covers pairs PAIRS[2i], PAIRS[2i+1]; the
# first two iterations take the direct-xp pairs (one-hot selector weights)
PFX = [0, 32, 1, 33]
PAIRS = PFX + list(range(2, 32)) + list(range(34, 64))
ACT_EVICT_ITERS = (30,)


# ----------------------------------------------------------------- host math
def _piece_poly_coeffs(knots, i, ell):
    """Monomial coeffs (len 4) of the de Boor piece for element i, interval
    ell in [3,6] -- replicates the reference recursion, fit exactly in f64."""
    k = DEGREE
    seg = knots[i:i + k + 2]
    T = np.concatenate([np.full(k, seg[0] - 1.0), seg, np.full(k, seg[-1] + 1.0)])

    def eval_at(x):
        res = [np.float64(1.0)] + [np.float64(0.0)] * k
        for j in range(1, k + 1):
            hh = list(res[:j])
            res[0] = np.float64(0.0)
            for n in range(1, j + 1):
                tb, ta = T[ell + n], T[ell + n - j]
                den = tb - ta
                w = 0.0 if den == 0 else hh[n - 1] / den
                res[n - 1] = res[n - 1] + w * (tb - x)
                res[n] = w * (x - ta)
        return res[2 * k - ell]

    xs = np.linspace(-0.3, 1.3, 5)
    V = np.vander(xs, 4, increasing=True)
    return np.linalg.lstsq(V, np.array([eval_at(x) for x in xs]), rcond=None)[0]


def build_tables(knots):
    """A [4,60], J [56,60], row1 [60] for the truncated-power form (xn basis)."""
    knots = np.asarray(knots, np.float64)
    P = [[_piece_poly_coeffs(knots, i, p + 3) for p in range(4)] for i in range(NB)]

    def p_of(s, i):
        return int(np.clip(s - i - 1, 0, 3))

    A = np.zeros((4, NB))
    for i in range(NB):
        A[:, i] = P[i][p_of(4, i)]

    ms = list(range(4, 60))
    J = np.zeros((len(ms), NB))
    for f, m in enumerate(ms):
        for i in range(NB):
            pb, pa = p_of(m, i), p_of(m + 1, i)
            if pa != pb:
                J[f, i] = (P[i][pa] - P[i][pb])[3]

    # reference row at xn == 1.0 exactly: searchsorted gives s = 64 there,
    # so every column evaluates its piece p=3 at 1.0 (a genuine jump for the
    # right-boundary columns); patched on the host for the (rare) argmax hits
    row1 = np.array([np.polyval(P[i][3][::-1], 1.0) for i in range(NB)])
    return A, J, row1


def _shift_poly(c, h):
    """coeffs of p(y + h) given coeffs c of p(x), low->high, exact in f64."""
    from math import comb
    out = np.zeros_like(c)
    for q in range(4):
        for r in range(q + 1):
            out[r] += c[q] * comb(q, r) * h ** (q - r)
    return out


def _make_const_arrays(knots):
    A_xn, J, row1 = build_tables(knots)
    # re-express the base cubic in y = xn - CENTER
    A = np.stack([_shift_poly(A_xn[:, i], CENTER) for i in range(NB)], 1)
    kaps = np.asarray(knots, np.float64)[4:60] - CENTER
    CUBE4 = np.stack([-kaps**3, 3 * kaps**2, -3 * kaps, np.ones(NF)], 0)

    # MM1 weights cu6 [6, 120]: rows 0-2 = (y, y^2, y^3) coeffs for half A
    # (cols 0-59), rows 3-5 for half B (cols 60-119). Cols 0-55 produce the
    # non-constant part of (y - kap')^3; cols 56-59 pass y-powers through.
    c3 = np.zeros((3, NB), np.float64)
    c3[:, :NF] = CUBE4[1:4, :]
    for q in range(1, 4):
        c3[q - 1, NF + q] = 1.0
    cu6 = np.zeros((6, 2 * NB), np.float64)
    for h in (0, 1):
        for q in (0, 1, 2):
            cu6[2 * q + h, h * NB:(h + 1) * NB] = c3[q, :]

    # clamp bias [120, 1]: constant terms; power rows get nonneg shifts so a
    # plain Relu is exact on every row.
    bias = np.zeros((2 * NB, 1), np.float64)
    for h in (0, NB):
        bias[h:h + NF, 0] = CUBE4[0, :]      # -kap'^3
        bias[h + NF + 0, 0] = 1.0            # ones row
        bias[h + NF + 1, 0] = SHIFT1         # y + 0.5 = xn in [0, 1)
        bias[h + NF + 2, 0] = 0.0            # y^2 >= 0
        bias[h + NF + 3, 0] = SHIFT3         # y^3 + 0.125 >= 0

    # MM2 weights: rows 0-55 = J band, rows 56-59 = base cubic A with the
    # shift corrections folded into the ones row.
    w2s = np.zeros((NB, NB), np.float64)
    w2s[:NF, :] = J
    w2s[NF + 0, :] = A[0, :] - SHIFT1 * A[1, :] - SHIFT3 * A[3, :]
    w2s[NF + 1, :] = A[1, :]
    w2s[NF + 2, :] = A[2, :]
    w2s[NF + 3, :] = A[3, :]
    w2 = np.zeros((2 * NB, 2 * NB), np.float64)
    w2[:NB, :NB] = w2s
    w2[NB:, NB:] = w2s
    # fp16 scaling: stack rows x16 (via cu6/bias), W2 /16 -- keeps every
    # fp16 value well inside [6e-5, 65504] while preserving the product.
    SC = 16.0
    cu6h = (cu6 * SC).astype(np.float16)
    w2h = (w2 / SC).astype(np.float16)
    consts = np.zeros((120, 121), np.float32)
    consts[0:120, 0:1] = bias * SC
    consts[0:6, 1:61] = cu6h.view(np.float32)
    consts[0:120, 61:121] = w2h.view(np.float32)

    # one-hot K=128 row-selector weights for the direct-xp prefix pairs:
    # piece (k, q) at cols (3k+q)*120; pair PFX[k] has halves (a, 64+a), so
    # rows a and 64+a carry cu6h rows 2q (half A) and 2q+1 (half B)
    consts2 = np.zeros((98, 12 * 120), np.float16)
    for k, a in enumerate(PFX):
        for q in range(3):
            c0 = (3 * k + q) * 120
            consts2[a, c0:c0 + 120] = cu6h[2 * q]
            consts2[64 + a, c0:c0 + 120] = cu6h[2 * q + 1]
    return consts, consts2, row1


# -------------------------------------------------------------- bass program
_CACHE = {}


def _build_nc():
    import concourse.tile as tile
    import concourse.bass as bassmod
    from concourse import bacc, mybir

    bass_isa = bassmod.bass_isa
    f32 = mybir.dt.float32
    bf16 = mybir.dt.bfloat16
    f16 = mybir.dt.float16
    W = TILE_W

    nc = bacc.Bacc("TRN2", target_bir_lowering=False, debug=False)
    x_full = nc.declare_dram_parameter("x_full", [128, FULL_COLS], f32, isOutput=False)
    x_shard = nc.declare_dram_parameter("x_shard", [128, W], f32, isOutput=False)
    consts_d = nc.declare_dram_parameter("consts", [120, 121], f32, isOutput=False)
    consts2_d = nc.declare_dram_parameter("consts2", [98, 12 * 120], f16, isOutput=False)
    out_t = nc.declare_dram_parameter("out_t", [2 * NB, HALF], bf16, isOutput=True)

    with tile.TileContext(nc) as tc:
        with (
            tc.tile_pool(name="big", bufs=1) as big_pool,
            tc.tile_pool(name="consts", bufs=1) as const_pool,
            tc.tile_pool(name="stack", bufs=6) as stack_pool,
            tc.tile_pool(name="stage", bufs=8) as stage_pool,
            tc.tile_pool(name="psum", bufs=4, space="PSUM") as psum_pool,
            tc.tile_pool(name="tiny", bufs=1) as tiny_pool,
        ):
            # ---- all input DMAs share the sync queue: the 8 min/max chunks
            # first (they gate the whole pipeline), then shard and tables --
            # queue order keeps the 2MB stream unconte nded on the DMA bus
            xf = big_pool.tile([128, FULL_COLS], f32)
            CHS = [537] * 7 + [FULL_COLS - 7 * 537]
            NCH = len(CHS)
            MTAIL = 48
            pq = tiny_pool.tile([128, 2 * NCH], f32)
            gmax = tiny_pool.tile([1, NCH + 1], f32)
            chunks = []
            lo = 0
            for sz in CHS:
                chunks.append((lo, lo + sz))
                nc.sync.dma_start(xf[:, lo:lo + sz], x_full[:, lo:lo + sz])
                lo += sz

            xs = big_pool.tile([128, W], f32)
            nc.sync.dma_start(xs[:, :], x_shard[:, :])
            c2t = const_pool.tile([98, 12 * 120], f16)
            nc.sync.dma_start(c2t[:, :], consts2_d[:, :])
            ct = const_pool.tile([120, 121], f32)
            nc.sync.dma_start(ct[:, :], consts_d[:, :])
            bias = ct[0:120, 0:1]
            cu = ct[0:6, 1:61].bitcast(f16)
            w2t = ct[0:120, 61:121].bitcast(f16)

            ones = tiny_pool.tile([1, 128], f32)
            nc.vector.memset(ones[:, :], 1.0)
            # preload the ACT function table during the DMA stream so the
            # 1.3us LoadActFuncSet is off the first-clamp critical path
            actwarm = tiny_pool.tile([1, 1], f32)
            nc.scalar.activation(actwarm[0:1, 0:1], ones[0:1, 0:1],
                                 mybir.ActivationFunctionType.Relu)

            # ---- min/max reduces: DVE min partials + tail maxes, Pool
            # full-chunk maxes; tiny PE warmups after each chunk keep the
            # p-state clock running so the main loop starts at full speed
            n_pool = 0
            for ci, (lo, hi) in enumerate(chunks):
                mid = max(lo, hi - MTAIL)
                nc.vector.tensor_reduce(
                    pq[:, ci:ci + 1], xf[:, lo:hi], mybir.AxisListType.X,
                    mybir.AluOpType.min)
                nc.vector.tensor_reduce(
                    pq[:, NCH + ci:NCH + ci + 1], xf[:, mid:hi],
                    mybir.AxisListType.X, mybir.AluOpType.max)
                if mid > lo:
                    nc.gpsimd.tensor_reduce(
                        gmax[0:1, n_pool:n_pool + 1], xf[:, lo:mid],
                        mybir.AxisListType.XYZWC, mybir.AluOpType.max)
                    n_pool += 1
                warm = psum_pool.tile([128, 2], f32, tag="gp", name="warm")
                nc.tensor.matmul(warm[:, 0:1], ones[:, :], pq[0:1, ci:ci + 1])

            # ---- finalize: pack (-min, max) partials into [128, 2], merge
            # Pool's scalar maxes, partition_all_reduce, then derive the
            # per-partition (inv, q) scalars for y = x*inv + q on DVE only
            pk = tiny_pool.tile([128, 2], f32)
            nc.vector.tensor_reduce(
                pk[:, 0:1], pq[:, 0:NCH], mybir.AxisListType.X,
                mybir.AluOpType.min, negate=True)
            nc.vector.tensor_reduce(
                pk[:, 1:2], pq[:, NCH:2 * NCH], mybir.AxisListType.X,
                mybir.AluOpType.max)
            gm0 = tiny_pool.tile([1, 1], f32)
            nc.vector.tensor_reduce(
                gm0[0:1, 0:1], gmax[0:1, 0:n_pool], mybir.AxisListType.X,
                mybir.AluOpType.max)
            nc.vector.tensor_max(pk[0:1, 1:2], pk[0:1, 1:2], gm0[0:1, 0:1])
            par = tiny_pool.tile([128, 2], f32)   # (-min, max) everywhere
            nc.gpsimd.partition_all_reduce(
                par[:, :], pk[:, :], 128, bass_isa.ReduceOp.max)
            sc = tiny_pool.tile([128, 3], f32)    # [span+eps, inv, q]
            nc.vector.tensor_scalar(
                sc[:, 0:1], par[:, 1:2], par[:, 0:1], 1e-8,
                mybir.AluOpType.add, mybir.AluOpType.add)
            nc.vector.reciprocal(sc[:, 1:2], sc[:, 0:1])
            # q = (-min - 0.5*span) * inv  (y = x*inv + q)
            nc.vector.tensor_scalar(
                sc[:, 2:3], sc[:, 0:1], -0.5, par[:, 0:1],
                mybir.AluOpType.mult, mybir.AluOpType.add)
            nc.vector.tensor_mul(sc[:, 2:3], sc[:, 2:3], sc[:, 1:2])

            # ---- power table xp [128, 3*490]; each xr6 DMA issues as soon
            # as its power row is ready (three queues), overlapping the rest
            # of the chain; tiny PE warmups keep the p-state clock alive
            xp = big_pool.tile([128, 3 * W], f16)
            xr6 = big_pool.tile([6, N_PAIRS * W], f16)
            nc.vector.tensor_scalar(
                xp[:, 0:W], xs[:, :], sc[:, 1:2], sc[:, 2:3],
                mybir.AluOpType.mult, mybir.AluOpType.add)
            nc.gpsimd.dma_start(xr6[0:2, :], xp[:, 0:W])
            warm = psum_pool.tile([128, 2], f32, tag="gp", name="warm")
            nc.tensor.matmul(warm[:, 0:1], ones[:, :], xp[0:1, 0:2].bitcast(f32))
            nc.vector.tensor_mul(xp[:, W:2 * W], xp[:, 0:W], xp[:, 0:W])
            nc.sync.dma_start(xr6[2:4, :], xp[:, W:2 * W])
            warm = psum_pool.tile([128, 2], f32, tag="gp", name="warm")
            nc.tensor.matmul(warm[:, 0:1], ones[:, :], xp[0:1, W:W + 2].bitcast(f32))
            nc.vector.tensor_mul(xp[:, 2 * W:3 * W], xp[:, W:2 * W], xp[:, 0:W])
            nc.scalar.dma_start(xr6[4:6, :], xp[:, 2 * W:3 * W])
            for kk in (1, 3):
                warm = psum_pool.tile([128, 4], f32, tag="gp", name="warm")
                nc.tensor.matmul(warm[0:2, 0:4], ct[0:kk, 0:1].bitcast(f16),
                                 xp[0:kk, 0:4])

            # ---- main pipeline: 32 iterations of 2 pairs each. PE runs 2
            # iterations ahead on MM1; MM2 reuses MM1's PSUM banks (legal:
            # MM2 already waits on the clamp that read them), so the psum
            # pool holds 4 iterations in flight on the 8 banks. Iterations
            # 0-1 run MM1 straight off xp (no xr6 dependency).
            BANK = 512
            n_iters = N_PAIRS // 2
            gps = {}
            stks = {}
            stages = {}

            def mm1(it):
                gp = psum_pool.tile([2 * NB, 2 * BANK], f32, name="gp")
                gps[it] = gp
                if 2 * it + 1 < len(PFX):
                    # direct-xp: three accumulating one-hot matmuls per
                    # pair; the base-0 K window only needs to reach the
                    # pair's upper half (64+a), keeping consts2 small --
                    # its DMA gates the prefix start. q-major so the y-
                    # and y^2-gated pieces run while y^3 is computed
                    for q in range(3):
                        for j, col in ((0, 0), (1, BANK)):
                            a = PAIRS[2 * it + j]
                            kk = 66 if a < 32 else 98
                            c0 = (3 * (2 * it + j) + q) * 120
                            nc.tensor.matmul(
                                gp[:, col:col + W],
                                c2t[0:kk, c0:c0 + 120],
                                xp[0:kk, q * W:(q + 1) * W],
                                start=(q == 0), stop=(q == 2))
                    return
                for j, col in ((0, 0), (1, BANK)):
                    p = PAIRS[2 * it + j]
                    nc.tensor.matmul(gp[:, col:col + W], cu[:, :],
                                     xr6[:, p * W:(p + 1) * W])

            def gview(it):
                return gps[it][:, :].rearrange(
                    "r (p c) -> r p c", c=BANK)[:, :, 0:W]

            def clamp(it):
                stk = stack_pool.tile([2 * NB, 2 * W], f16, name="stk")
                stks[it] = stk
                stkv = stk[:, :].rearrange("r (p c) -> r p c", c=W)
                gpv = gview(it)
                # stack = Relu(G + bias) on ACT (GPSIMD cannot access PSUM,
                # so only ACT/DVE can touch the matmul results)
                nc.scalar.activation(
                    stkv[:, :, 0:W], gpv[:, :, 0:W],
                    mybir.ActivationFunctionType.Relu, bias=bias[:, 0:1])

            def mm2(it):
                gp, stk = gps[it], stks[it]
                nc.tensor.matmul(gp[:, 0:W], w2t[:, :], stk[:, 0:W])
                nc.tensor.matmul(gp[:, BANK:BANK + W], w2t[:, :],
                                 stk[:, W:2 * W])

            # lookahead: clamps run 2 iterations ahead and MM2 one ahead, so
            # each evict's input is ready a full iteration before the evict
            # stream reaches it -- DVE never stalls, even across the two
            # iterations whose evict runs on ACT
            mm1(0)
            mm1(1)
            clamp(0)
            clamp(1)
            mm2(0)
            for it in range(n_iters):
                solo = it >= n_iters - 4      # tail: per-iter stages spread the bus load
                if solo:
                    stage = stage_pool.tile([2 * NB, 2 * W], bf16, name="stage")
                    sl = 0
                else:
                    gi = it // 2
                    if it % 2 == 0:
                        stages[gi] = stage_pool.tile(
                            [2 * NB, 4 * W], bf16, name="stage")
                    stage = stages[gi]
                    sl = (it % 2) * 2 * W

                if it + 2 < n_iters:
                    mm1(it + 2)
                    clamp(it + 2)
                if it + 1 < n_iters:
                    mm2(it + 1)

                gpv = gview(it)
                stv = stage[:, sl:sl + 2 * W].rearrange("r (p c) -> r p c", c=W)
                # evict PSUM -> bf16 stage on DVE; the iteration-30 evict
                # runs on ACT after its clamps finish (mid-stream ACT evicts
                # stall the DVE stream; splitting the final evict across
                # engines also measured slower)
                if it in ACT_EVICT_ITERS:
                    nc.scalar.activation(
                        stv[:, :, 0:W], gpv[:, :, 0:W],
                        mybir.ActivationFunctionType.Copy)
                else:
                    nc.vector.tensor_scalar(
                        stv[:, :, 0:W], gpv[:, :, 0:W], 0.0, None,
                        mybir.AluOpType.add)
                if solo:
                    nc.sync.dma_start(
                        out_t[:, it * 2 * W:(it + 1) * 2 * W], stage[:, :])
                elif it % 2 == 1:
                    gcol = gi * 4 * W
                    nc.sync.dma_start(
                        out_t[:, gcol:gcol + 4 * W], stage[:, :])

    nc.compile()
    return nc


# ------------------------------------------------------------------- driver
def _run(in_maps, trace=False):
    from concourse.bass_utils import run_bass_kernel_spmd

    if "nc" not in _CACHE:
        _CACHE["nc"] = _build_nc()
    return run_bass_kernel_spmd(
        _CACHE["nc"], in_maps, list(range(N_CORES)), trace=trace
    )


def _default_knots():
    inner = np.linspace(0.0, 1.0, NUM_KNOTS - 2 * DEGREE)
    return np.concatenate(
        [np.zeros(DEGREE), inner, np.ones(DEGREE)]
    ).astype(np.float32)


def kernel(x, knots=None, degree=None, _trace=False, _return_results=False, **_):
    x = np.asarray(x, np.float32).reshape(-1)
    assert x.size == N_POINTS
    if knots is None:
        knots = _default_knots()
    consts, consts2, row1 = _make_const_arrays(np.asarray(knots, np.float64))

    xf = np.empty(FULL_PAD, np.float32)
    xf[:N_POINTS] = x
    xf[N_POINTS:] = x[0]
    xf = xf.reshape(128, FULL_COLS)

    in_maps = []
    for c in range(N_CORES):
        sh = np.empty(SHARD_PAD, np.float32)
        sh[:SHARD] = x[c * SHARD:(c + 1) * SHARD]
        sh[SHARD:] = x[c * SHARD]
        in_maps.append({
            "x_full": xf,
            "x_shard": sh.reshape(128, TILE_W),
            "consts": consts,
            "consts2": consts2,
        })

    res = _run(in_maps, trace=_trace)
    order = np.asarray(PAIRS)
    out = np.empty((N_POINTS, NB), np.float32)
    full = np.empty((128, TILE_W, NB), np.float32)
    for c in range(N_CORES):
        ot = np.asarray(res.results[c]["out_t"], np.float32)  # [120, 31360]
        # col block k of out_t holds pair PAIRS[k]: rows 0:60 -> x block
        # PAIRS[k], rows 60:120 -> x block 64+PAIRS[k]
        otv = ot.reshape(2 * NB, N_PAIRS, TILE_W)
        full[order] = np.transpose(otv[:NB], (1, 2, 0))
        full[order + 64] = np.transpose(otv[NB:], (1, 2, 0))
        out[c * SHARD:(c + 1) * SHARD, :] = \
            full.reshape(SHARD_PAD, NB)[:SHARD]

    # boundary fixup: at xn == 1.0 exactly the reference jumps to the
    # degenerate right-end pieces (s = 64); patch those rows exactly
    mn, mx = x.min(), x.max()
    xn = (x - mn) / ((mx - mn) + np.float32(1e-8))
    at_one = np.nonzero(xn == np.float32(1.0))[0]
    if at_one.size:
        out[at_one, :] = row1.astype(np.float32)[None, :]

    if _return_results:
        return out, res
    return out
